# revision 10
# baseline (speedup 1.0000x reference)
"""Compressed (block-stride) attention on 8 Trainium2 NeuronCores.

Shards the 32 (batch, head) pairs across 8 cores (4 per core); k/v and the
block-stride mask are per-shard, no cross-core communication.

Layout: per (b,h) unit, queries are processed in 4 batches of 1024 rows. A
batch is 8 interleaved 128-row sub-tiles: SBUF partition r of sub-tile s holds
query row 1024*bt + 8*r + s, so batched q/p/o DMAs move 4-8KB contiguous
chunks per partition (DMA instruction issue on the sync engine costs ~650ns
flat, so few big DMAs beat many small ones).

Per sub-tile:  s = qT.T @ kT  (fp32r matmuls, PSUM), boundary mask added by
DVE from a tiny shift-invariant [128,8,258] table, exp on ACT (fp32r out),
p~ transposed on PE, PV matmul against v augmented with a ones column (row
sums come out as column 128 of o for free), reciprocal on DVE, p normalized
on GpSimd, o scaled+copied on ACT. Fully-masked columns (>= 8t+7 per 128-row
tile) produce exact zeros and are never written -- outputs are pre-zeroed.
"""

import os
import sys

import numpy as np

for _p in ("/opt/trn_rl_repo", "/root/.axon_site/_ro/trn_rl_repo"):
    if os.path.isdir(_p) and _p not in sys.path:
        sys.path.insert(0, _p)

import concourse.bacc as bacc
import concourse.mybir as mybir
import concourse.tile as tile
from concourse.bass_utils import run_bass_kernel_spmd

B, M, QH, D = 2, 4096, 16, 192
N, VD = 255, 128
NP = 256                      # n padded to 256 (fp32r QK wants free >= 256)
VDA = VD + 4                  # v + ones column (128) + zero pad to keep fp32r ISA happy
NCORES = 8
U = (B * QH) // NCORES        # 4 (b,h) units per core
NBT = 4                       # batches per unit
NS = 8                        # interleaved sub-tiles per batch
KSZ, STR = 32, 16
SCALE = float(D) ** -0.5
# power of two: scale*MASKVAL is exactly fp32-representable, so the batch-0
# max-subtraction cancels exactly even in the ACT engine's extended-precision
# multiply-add (exp(0)=1 for fully-masked rows -> uniform 1/255, as reference)
MASKVAL = -(2.0 ** 100)
F32 = mybir.dt.float32
F32R = mybir.dt.float32r
AX = mybir.AxisListType.X
EXP = mybir.ActivationFunctionType.Exp
ADD = mybir.AluOpType.add

_cache: dict = {}


def _build():
    nc = bacc.Bacc()
    qT = nc.dram_tensor("qT", [U, D, M], F32R, kind="ExternalInput")
    kT = nc.dram_tensor("kT", [U, D, NP], F32R, kind="ExternalInput")
    vv = nc.dram_tensor("v", [U, N, VDA], F32R, kind="ExternalInput")
    mb = nc.dram_tensor("mb", [128, NS, 258], F32, kind="ExternalInput")
    idr = nc.dram_tensor("identr", [128, 128], F32R, kind="ExternalInput")
    od = nc.dram_tensor("o", [U, M, VD], F32, kind="ExternalOutput")
    pd = nc.dram_tensor("p", [U, M, N], F32, kind="ExternalOutput")

    with tile.TileContext(nc) as tc:
        with tc.tile_pool(name="const", bufs=1) as cpool, \
             tc.tile_pool(name="kv", bufs=2) as kvpool, \
             tc.tile_pool(name="qb", bufs=2) as qpool, \
             tc.tile_pool(name="ptb", bufs=2) as ptpool, \
             tc.tile_pool(name="pnb", bufs=2) as pnpool, \
             tc.tile_pool(name="ob", bufs=2) as opool, \
             tc.tile_pool(name="pts", bufs=3) as ptspool, \
             tc.tile_pool(name="st", bufs=3) as stpool, \
             tc.tile_pool(name="ps_s", bufs=3, space="PSUM") as ps_s, \
             tc.tile_pool(name="ps_pt", bufs=2, space="PSUM") as ps_pt, \
             tc.tile_pool(name="ps_o", bufs=2, space="PSUM") as ps_o:

            identr = cpool.tile([128, 128], F32R, tag="identr")
            nc.sync.dma_start(out=identr, in_=idr[:, :])
            mb_sb = cpool.tile([128, NS, 258], F32, tag="mb")
            nc.sync.dma_start(out=mb_sb, in_=mb[:, :, :])

            for u in range(U):
                kT_sb = kvpool.tile([96, 2, NP], F32R, tag="kT")
                nc.sync.dma_start(
                    out=kT_sb, in_=kT[u].rearrange("(c p) n -> p c n", p=96)
                )
                v_sb = kvpool.tile([128, 2, VDA], F32R, tag="v")
                nc.sync.dma_start(out=v_sb[:, 0, :], in_=vv[u, 0:128, :])
                nc.sync.dma_start(out=v_sb[0:127, 1, :], in_=vv[u, 128:255, :])

                for bt in range(NBT):
                    mrow = 1024 * bt
                    c0 = max(0, 64 * bt - 2)          # first maskable column
                    u0 = c0 - (64 * bt - 2)           # its offset in mb table
                    nw = min(N, 64 * bt + 63)         # cols beyond are all-masked
                    if bt == 0:
                        nw = N                        # rows<31 need full width
                    W = nw - c0
                    ck0 = min(nw, 128)
                    ck1 = max(0, nw - 128)

                    q_sb = qpool.tile([96, 2, NS, 128], F32R, tag="q")
                    nc.sync.dma_start(
                        out=q_sb,
                        in_=qT[u, :, mrow:mrow + 1024].rearrange(
                            "(c p) (s m) -> p c s m", p=96, s=NS),
                    )
                    pt_sb = ptpool.tile([128, NS, NP], F32R, tag="pt")
                    pn_sb = pnpool.tile([128, NS, NP], F32R, tag="pn")
                    o_sb = opool.tile([128, NS, VD], F32, tag="ob")
                    rs_sb = stpool.tile([128, NS], F32, tag="rs")

                    for s in range(NS):
                        sreg = ps_s.tile([128, NP], F32, tag="s")
                        nc.tensor.matmul(sreg, q_sb[:, 0, s, :],
                                         kT_sb[:, 0, :],
                                         start=True, stop=False)
                        nc.tensor.matmul(sreg, q_sb[:, 1, s, :],
                                         kT_sb[:, 1, :],
                                         start=False, stop=True)
                        nc.vector.tensor_tensor(
                            out=sreg[:, c0:c0 + W], in0=sreg[:, c0:c0 + W],
                            in1=mb_sb[:, s, u0:u0 + W], op=ADD)

                        if bt == 0:
                            mx = stpool.tile([128, 1], F32, tag="mx")
                            nc.vector.reduce_max(mx, sreg[:, 0:N], axis=AX)
                            bias = stpool.tile([128, 1], F32, tag="bias")
                            nc.scalar.mul(bias, mx, -SCALE)
                            nc.scalar.activation(pt_sb[:, s, 0:nw],
                                                 sreg[:, 0:nw], EXP,
                                                 bias=bias, scale=SCALE)
                        else:
                            nc.scalar.activation(pt_sb[:, s, 0:nw],
                                                 sreg[:, 0:nw], EXP,
                                                 bias=0.0, scale=SCALE)

                        pT_ps = ps_pt.tile([128, 2, 128], F32R, tag="pT")
                        nc.tensor.transpose(pT_ps[0:ck0, 0, :],
                                            pt_sb[:, s, 0:ck0], identr)
                        if ck1:
                            nc.tensor.transpose(pT_ps[0:ck1, 1, :],
                                                pt_sb[:, s, 128:128 + ck1],
                                                identr)
                        pT_sb = ptspool.tile([128, 2, 128], F32R, tag="pTs")
                        nch = 2 if ck1 else 1
                        nc.vector.tensor_copy(pT_sb[:, 0:nch, :],
                                              pT_ps[:, 0:nch, :])

                        oreg = ps_o.tile([128, VDA], F32, tag="o")
                        nc.tensor.matmul(oreg, pT_sb[0:ck0, 0, :],
                                         v_sb[0:ck0, 0, :],
                                         start=True, stop=(ck1 == 0))
                        if ck1:
                            nc.tensor.matmul(oreg, pT_sb[0:ck1, 1, :],
                                             v_sb[0:ck1, 1, :],
                                             start=False, stop=True)

                        # row sums of p~ sit in column VD (ones column of v)
                        nc.vector.reciprocal(rs_sb[:, s:s + 1], oreg[:, VD:VD + 1])
                        nc.scalar.mul(o_sb[:, s, :], oreg[:, 0:VD],
                                      rs_sb[:, s:s + 1])
                        nc.gpsimd.tensor_scalar_mul(
                            pn_sb[:, s, 0:nw], pt_sb[:, s, 0:nw],
                            rs_sb[:, s:s + 1])
                        if bt == 0:
                            # o rows with query index < 31 are zeroed
                            rmax = (30 - s) // NS + 1
                            nc.gpsimd.memset(o_sb[0:rmax, s, :], 0.0)

                    nc.sync.dma_start(
                        out=od[u, mrow:mrow + 1024, :].rearrange(
                            "(r s) d -> r s d", s=NS),
                        in_=o_sb)
                    if bt == 0:
                        # dense window + full-width rows i < 32 (31 uniform
                        # rows plus row 31 whose tail is exact zeros anyway)
                        nc.sync.dma_start(
                            out=pd[u, mrow:mrow + 1024, 0:63].rearrange(
                                "(r s) n -> r s n", s=NS),
                            in_=pn_sb[:, :, 0:63].bitcast(F32))
                        nc.sync.dma_start(
                            out=pd[u, mrow:mrow + 32, 63:N].rearrange(
                                "(r s) n -> r s n", s=NS),
                            in_=pn_sb[0:4, :, 63:N].bitcast(F32))
                    else:
                        nc.sync.dma_start(
                            out=pd[u, mrow:mrow + 1024, 0:nw].rearrange(
                                "(r s) n -> r s n", s=NS),
                            in_=pn_sb[:, :, 0:nw].bitcast(F32))

    nc.compile()
    return nc


def _host_mask() -> np.ndarray:
    # mb[r, s, t] = MASKVAL where query row 8r+s of a batch is masked at
    # boundary column offset t (column c = 64*bt - 2 + t); shift-invariant
    r = np.arange(128)[:, None, None]
    s = np.arange(NS)[None, :, None]
    t = np.arange(258)[None, None, :]
    return np.where(8 * r + s < 16 * t - 1, np.float32(MASKVAL),
                    np.float32(0.0)).astype(np.float32)


def _make_in_maps(q, k, v):
    # [b,h]-major shards; q/k contraction-dim-major; q column order matches the
    # kernel's interleave: column 1024*bt + 128*s + r <-> query row 1024*bt+8r+s
    qTh = q.transpose(0, 2, 3, 1).reshape(B * QH, D, NBT, 128, NS)
    qTh = np.ascontiguousarray(qTh.transpose(0, 1, 2, 4, 3)).reshape(
        B * QH, D, M)
    kTh = np.zeros((B * QH, D, NP), dtype=np.float32)
    kTh[:, :, 0:N] = k.transpose(0, 2, 3, 1).reshape(B * QH, D, N)
    vh = np.zeros((B * QH, N, VDA), dtype=np.float32)
    vh[:, :, VD] = 1.0
    vh[:, :, 0:VD] = v.transpose(0, 2, 1, 3).reshape(B * QH, N, VD)
    mask = _host_mask()
    ident = np.eye(128, dtype=np.float32)

    in_maps = []
    for c in range(NCORES):
        sl = slice(c * U, (c + 1) * U)
        in_maps.append({
            "qT": np.ascontiguousarray(qTh[sl]),
            "kT": np.ascontiguousarray(kTh[sl]),
            "v": np.ascontiguousarray(vh[sl]),
            "mb": mask,
            "identr": ident,
        })
    return in_maps


def kernel(q, k, v, real_length, kernel_size, stride):
    q = np.asarray(q, dtype=np.float32)
    k = np.asarray(k, dtype=np.float32)
    v = np.asarray(v, dtype=np.float32)
    assert q.shape == (B, M, QH, D) and k.shape == (B, N, QH, D)
    assert int(real_length) == M and int(kernel_size) == KSZ and int(stride) == STR

    if "nc" not in _cache:
        _cache["nc"] = _build()
    nc = _cache["nc"]
    in_maps = _make_in_maps(q, k, v)

    res = run_bass_kernel_spmd(nc, in_maps, list(range(NCORES)))

    o_sh = np.stack([res.results[c]["o"] for c in range(NCORES)])  # [8, 4, M, VD]
    p_sh = np.stack([res.results[c]["p"] for c in range(NCORES)])  # [8, 4, M, N]
    o = o_sh.reshape(B, QH, M, VD).transpose(0, 2, 1, 3)           # [b, m, h, vd]
    p = p_sh.reshape(B, QH, M, N)                                  # [b, h, m, n]
    return (np.ascontiguousarray(o), np.ascontiguousarray(p))
